# revision 1
# baseline (speedup 1.0000x reference)
"""Trainium2 Bass kernel for ChamferLossSplitPID.

Contract: kernel(**inputs) takes the FULL inputs (target/reco [64,512,4] f32,
in_pid/out_pid [64,512] i32) and returns the full output (loss_nonzero,
loss_zero) as float32 scalars, matching reference().

Strategy (8 NeuronCores, data-parallel over batch, 8 batches per core):
  dist^2[n,m] = |t_n|^2 + |r_m|^2 - 2 t.r computed on the PE as a K=16
  split-bf16 matmul (a.b ~ ahi.bhi + ahi.blo + alo.bhi, norm terms split
  hi/lo too; ~1e-5 relative accuracy at full bf16 speed). The "other side"
  points are permuted into 4 pid groups padded to a fixed S=130 columns
  (pad cols produce dist^2 = 2^27, never a min winner). Because sqrt is
  monotone, per-pid mins are taken on dist^2: all 4 matmuls of a (batch,
  row-chunk) land in one 4-bank PSUM tile at a uniform 256-element slot
  stride, so ONE 3D-AP DVE reduce yields both directions x 4 group minima.
  Only the [128, 16x16] minima get relu+sqrt; a host-built 0/1 row mask
  selects the rows of each pid's sum, and partition sums go through a
  GpSimd all-reduce (mid-stream pieces) / PE ones-matmul (final piece).
  Per-pid norm sums run as one 72-row masked multiply+reduce. The tiny
  O(B*pid) epilogue (counts, divisions, empty-group branches, means) runs
  on the host, as does all layout prep (permutation, hi/lo splits, masks).

The emitted IR is input-value-independent (fixed group stride S), so one
SPMD program serves all 8 cores. S is bumped automatically if some pid
group exceeds it (recompile, still correct for any input).

Measured: ~62.6 us on hardware per core (8 cores run concurrently),
relative error vs the fp32 reference ~3e-6.
"""

import sys

sys.path.insert(0, "/opt/trn_rl_repo")

import numpy as np

from concourse import bacc, bass, bass_isa, mybir, tile
from concourse.bass_utils import run_bass_kernel_spmd

B, N, D = 64, 512, 4
NCORES = 8
BL = B // NCORES          # batches per core
P = 128                   # partitions
NCH = N // P              # row chunks per batch
NPID = 4                  # nonzero pid classes
BIG = float(2 ** 27)      # pad-column dist^2 (exact in bf16)
KROWS = 16                # split-bf16 contraction rows
F32 = mybir.dt.float32
BF16 = mybir.dt.bfloat16

_PROGRAM_CACHE = {}


def _build_program(S: int):
    """Emit the SPMD Bass program for group stride S. Value-independent."""
    COLS = NPID * S           # padded columns per batch per direction
    HALF = COLS // 2          # one matmul = 2 pid groups (<=512 psum floats)
    nc = bacc.Bacc(None)

    # lhsT and rhs for one direction share one tensor/DMA so the first
    # consuming Matmult carries a single sync wait (PE LW allows only one).
    d_ab1 = nc.dram_tensor("ab1", [BL, KROWS, N + COLS], BF16, kind="ExternalInput")
    d_ab2 = nc.dram_tensor("ab2", [BL, KROWS, N + COLS], BF16, kind="ExternalInput")
    d_rm = nc.dram_tensor("rmall", [P, 2 * BL * 16], F32, kind="ExternalInput")
    # norm sums in partition-parallel layout: row g*BL+b, g in (p1..p4 of
    # in_pid, p1..p4 of out_pid, p0 of out_pid)
    d_nrm = nc.dram_tensor("normrep", [9 * BL, N], F32, kind="ExternalInput")
    d_msk = nc.dram_tensor("mask72", [9 * BL, N], F32, kind="ExternalInput")
    d_sums = nc.dram_tensor("sums", [1, 2 * BL * NPID], F32, kind="ExternalOutput")
    d_ns = nc.dram_tensor("ns", [9 * BL, 1], F32, kind="ExternalOutput")

    with tile.TileContext(nc) as tc:
        with (
            tc.tile_pool(name="const", bufs=1) as const,
            tc.tile_pool(name="work", bufs=2) as work,
            tc.tile_pool(name="psum", bufs=2, space=bass.MemorySpace.PSUM) as psum,
        ):
            # one tile + one DMA per (dir, batch): matmuls for batch b start
            # as soon as its slice lands. dir-0 loads issue from the Sync
            # HWDGE, dir-1 from the Activation HWDGE (parallel issue).
            d_ab = [d_ab1, d_ab2]
            tAB = [[const.tile([KROWS, N + COLS], BF16, tag=f"ab{d}_{b}", name=f"tAB{d}_{b}")
                    for b in range(BL)] for d in range(2)]
            for b in range(BL):
                for d in range(2):
                    eng = nc.sync if d == 0 else nc.scalar
                    eng.dma_start(tAB[d][b][:], d_ab[d][b])
            tRM = const.tile([P, 2 * BL, NCH, NPID], F32, tag="rm")
            nc.sync.dma_start(tRM[:], d_rm[:].rearrange("p (a c q) -> p a c q", q=NPID, c=NCH))
            tNRM = const.tile([9 * BL, N], F32, tag="nrm")
            tMSK = const.tile([9 * BL, N], F32, tag="msk")
            nc.scalar.dma_start(tNRM[:], d_nrm[:])
            nc.scalar.dma_start(tMSK[:], d_msk[:])
            tONE = const.tile([P, 1], F32, tag="one")
            nc.vector.memset(tONE[:], 1.0)

            # per-pid norm sums, partition-parallel — emitted early so DVE
            # slots them into the pipeline ramp instead of the drain tail
            tNS = work.tile([9 * BL, 1], F32, tag="nsout")
            tmp72 = work.tile([9 * BL, N], F32, tag="tmp72")
            nc.vector.tensor_tensor(tmp72[:], tNRM[:], tMSK[:], op=mybir.AluOpType.mult)
            nc.vector.tensor_reduce(
                tNS[:], tmp72[:], axis=mybir.AxisListType.X, op=mybir.AluOpType.add)
            nc.sync.dma_start(d_ns[:], tNS[:])

            # minima of dist^2: [128, (b,dir), chunk, pid] (pid contiguous;
            # (b,dir) batch-major so each batch-half is a contiguous slice)
            tMS = const.tile([P, 2 * BL, NCH, NPID], F32, tag="ms")
            tSQ = const.tile([P, 2 * BL * NPID * NCH], F32, tag="sq")
            tMK = const.tile([P, 2 * BL * NPID * NCH], F32, tag="mk")
            tPR = const.tile([P, 2 * BL, NCH, NPID], F32, tag="pr")
            tSF = const.tile([1, 2 * BL * NPID], F32, tag="sf")

            NQ = 4  # tail pieces (2 batches each)

            def tail_half(h):
                # relu -> sqrt -> row-mask -> partition-sum -> chunk-sum for
                # dbs [h*4, h*4+4). Pieces 0-2 sum on idle GpSimd (mid-stream,
                # PSUM fully busy); the last piece uses a PE ones-matmul into
                # a now idle dist-pool PSUM slot (GpSimd is too slow there).
                w = 2 * BL * NPID * NCH // NQ  # 64 cols per piece
                sl = slice(h * w, (h + 1) * w)
                flat = tMS[:].rearrange("p a c q -> p (a c q)")[:, sl]
                nc.vector.tensor_scalar_max(flat, flat, 0.0)
                nc.scalar.activation(tSQ[:, sl], flat, mybir.ActivationFunctionType.Sqrt)
                nc.vector.tensor_tensor(
                    tMK[:, sl], tSQ[:, sl],
                    tRM[:].rearrange("p a c q -> p (a c q)")[:, sl],
                    op=mybir.AluOpType.mult,
                )
                hdb = 2 * BL // NQ  # dbs per piece
                if h < NQ - 1:
                    nc.gpsimd.partition_all_reduce(
                        tPR[:].rearrange("p a c q -> p (a c q)")[:, sl],
                        tMK[:, sl], P, bass_isa.ReduceOp.add,
                    )
                    srcrow = tPR[0:1, h * hdb:(h + 1) * hdb].rearrange("o a c q -> o a q c")
                else:
                    prow = psum.tile([1, hdb, NCH, NPID], F32, tag="dist", name="prow")
                    nc.tensor.matmul(
                        prow[:].rearrange("o a c q -> o (a c q)"),
                        tONE[:],
                        tMK[:, sl],
                        start=True,
                        stop=True,
                    )
                    srcrow = prow[:].rearrange("o a c q -> o a q c")
                nc.vector.tensor_reduce(
                    tSF[:].rearrange("o (a q) -> o a q", q=NPID)[:, h * hdb:(h + 1) * hdb, :],
                    srcrow,
                    axis=mybir.AxisListType.X,
                    op=mybir.AluOpType.add,
                )

            HB = BL  # dbs per half (4 batches x 2 dirs)
            for b in range(BL):
                for c in range(NCH):
                    if S <= 256:
                        # fast path: one 4-bank tile holds all 4 matmuls of
                        # (b,c); slot s = dr*4+j*2+g at uniform 256-elem
                        # stride, so ONE 3D-AP reduce covers both dirs x 4
                        # groups
                        pt = psum.tile([P, 8, 256], F32, tag="dist")
                        for dr in range(2):
                            for j in range(2):
                                nc.tensor.matmul(
                                    pt[:, dr * 4 + 2 * j : dr * 4 + 2 * j + 2, 0:S],
                                    tAB[dr][b][:, c * P : (c + 1) * P],
                                    tAB[dr][b][:, N + j * HALF : N + (j + 1) * HALF],
                                    start=True,
                                    stop=True,
                                )
                        nc.vector.tensor_reduce(
                            tMS[:, 2 * b : 2 * b + 2, c, :],
                            pt[:, :, 0:S],
                            axis=mybir.AxisListType.X,
                            op=mybir.AluOpType.min,
                        )
                    else:
                        # big-group fallback (S in (256, 512]): one matmul
                        # per pid group, 512-elem slots, per-direction tiles
                        for dr in range(2):
                            pt = psum.tile([P, 4, 512], F32, tag="dist")
                            for g in range(NPID):
                                nc.tensor.matmul(
                                    pt[:, g, 0:S],
                                    tAB[dr][b][:, c * P : (c + 1) * P],
                                    tAB[dr][b][:, N + g * S : N + (g + 1) * S],
                                    start=True,
                                    stop=True,
                                )
                            nc.vector.tensor_reduce(
                                tMS[:, 2 * b + dr, c, :],
                                pt[:, :, 0:S],
                                axis=mybir.AxisListType.X,
                                op=mybir.AluOpType.min,
                            )
                if b % 2 == 1 and b < BL - 1:
                    q = b // 2
                    tail_half(q)
                    w2 = 2 * BL * NPID // NQ
                    nc.sync.dma_start(
                        d_sums[:, q * w2 : (q + 1) * w2], tSF[:, q * w2 : (q + 1) * w2])
            tail_half(NQ - 1)
            nc.sync.dma_start(
                d_sums[:, (NQ - 1) * 2 * BL * NPID // NQ :],
                tSF[:, (NQ - 1) * 2 * BL * NPID // NQ :])



    nc.compile()
    return nc


def _get_program(S: int):
    if S not in _PROGRAM_CACHE:
        _PROGRAM_CACHE[S] = _build_program(S)
    return _PROGRAM_CACHE[S]


def _prep_inputs(target, reco, in_pid, out_pid, S):
    """Build per-core input maps. All heavy compute stays on device; this is
    O(B*N) metadata/layout prep (permutation, norms, masks, padding)."""
    COLS = NPID * S
    t = np.ascontiguousarray(np.asarray(target, dtype=np.float32))
    r = np.ascontiguousarray(np.asarray(reco, dtype=np.float32))
    ip = np.asarray(in_pid)
    op = np.asarray(out_pid)

    import ml_dtypes

    def split16(x):
        hi = x.astype(ml_dtypes.bfloat16).astype(np.float32)
        lo = (x - hi).astype(ml_dtypes.bfloat16).astype(np.float32)
        return hi, lo

    nt2 = (t * t).sum(-1)                      # [B,N]
    nr2 = (r * r).sum(-1)
    ones = np.ones((B, 1, N), np.float32)
    # split-bf16: a.b ~= ahi.bhi + ahi.blo + alo.bhi (lo.lo dropped, ~2^-16 rel)
    # lhsT rows: [(-2x)hi x4, (-2x)hi x4, (-2x)lo x4, |x|2hi, |x|2lo, 1, 1]
    # rhs rows:  [ yhi x4,     ylo x4,     yhi x4,    1,      1, |y|2hi, |y|2lo]
    def build_lhs(x, x2):
        m2hi, m2lo = split16(-2.0 * x.transpose(0, 2, 1))   # [B,4,N]
        x2hi, x2lo = split16(x2[:, None, :])                # [B,1,N]
        return np.concatenate(
            [m2hi, m2hi, m2lo, x2hi, x2lo, ones, ones], axis=1)  # [B,16,N]

    def build_rhs(x, x2, pid):
        rhs = np.zeros((B, KROWS, COLS), np.float32)
        rhs[:, 14, :] = BIG
        xhi, xlo = split16(x)                               # [B,N,4]
        y2hi, y2lo = split16(x2)                            # [B,N]
        for b in range(B):
            for p in range(1, 5):
                idx = np.nonzero(pid[b] == p)[0]
                k = len(idx)
                if k == 0:
                    continue
                c0 = (p - 1) * S
                rhs[b, 0:4, c0 : c0 + k] = xhi[b, idx].T
                rhs[b, 4:8, c0 : c0 + k] = xlo[b, idx].T
                rhs[b, 8:12, c0 : c0 + k] = xhi[b, idx].T
                rhs[b, 12:14, c0 : c0 + k] = 1.0
                rhs[b, 14, c0 : c0 + k] = y2hi[b, idx]
                rhs[b, 15, c0 : c0 + k] = y2lo[b, idx]
        return rhs

    lhs1 = build_lhs(t, nt2)
    lhs2 = build_lhs(r, nr2)
    rhs1 = build_rhs(r, nr2, op)   # dir0: rows=targets, cols=reco groups
    rhs2 = build_rhs(t, nt2, ip)   # dir1: rows=recos,  cols=target groups

    # row masks [B, 128, 2, 16]: (dir, b) -> col (pid-1)*4 + chunk
    pgrid = np.arange(1, 5)
    ohx = (ip.reshape(B, NCH, P)[:, :, :, None] == pgrid).astype(np.float32)  # [B,c,i,p]
    ohy = (op.reshape(B, NCH, P)[:, :, :, None] == pgrid).astype(np.float32)
    # -> [B, i(128), c, p]
    rm1 = ohx.transpose(0, 2, 1, 3)
    rm2 = ohy.transpose(0, 2, 1, 3)

    normt = np.sqrt(nt2).astype(np.float32)
    normr = np.sqrt(nr2).astype(np.float32)
    # 72-row layout: row g*BL+b; g 0-3: normt & in_pid==g+1;
    # g 4-7: normr & out_pid==g-3; g 8: normr & out_pid==0
    grp_norm = [normt] * 4 + [normr] * 5
    grp_mask = [(ip == p) for p in (1, 2, 3, 4)] + [(op == p) for p in (1, 2, 3, 4, 0)]

    in_maps = []
    for ci in range(NCORES):
        s = slice(ci * BL, (ci + 1) * BL)
        rm = np.zeros((P, 2 * BL, NCH, NPID), np.float32)
        rm[:, 0::2] = rm1[s].transpose(1, 0, 2, 3)
        rm[:, 1::2] = rm2[s].transpose(1, 0, 2, 3)
        ab1 = np.concatenate([lhs1[s], rhs1[s]], axis=2)  # [BL,16,N+COLS]
        ab2 = np.concatenate([lhs2[s], rhs2[s]], axis=2)
        normrep = np.concatenate([g[s] for g in grp_norm], axis=0)          # [72,N]
        mask72 = np.concatenate([g[s].astype(np.float32) for g in grp_mask], axis=0)
        in_maps.append({
            "ab1": np.ascontiguousarray(ab1.astype(ml_dtypes.bfloat16)),
            "ab2": np.ascontiguousarray(ab2.astype(ml_dtypes.bfloat16)),
            "rmall": np.ascontiguousarray(rm.reshape(P, 2 * BL * 16)),
            "normrep": np.ascontiguousarray(normrep),
            "mask72": np.ascontiguousarray(mask72),
        })
    return in_maps


def _epilogue(sums_all, ns_all, in_pid, out_pid):
    """Tiny O(B*pid) final combination, mirrors reference()'s branch logic."""
    ip = np.asarray(in_pid)
    op = np.asarray(out_pid)
    sum_xy = np.zeros((B, 5))
    sum_yx = np.zeros((B, 5))
    only_x = np.zeros((B, 5))
    only_y = np.zeros((B, 5))
    zerosum = np.zeros(B)
    for ci in range(NCORES):
        srow = sums_all[ci].reshape(BL, 2, NPID)
        nsrow = ns_all[ci]
        for lb in range(BL):
            b = ci * BL + lb
            sum_xy[b, 1:5] = srow[lb, 0]
            sum_yx[b, 1:5] = srow[lb, 1]
            ns72 = nsrow.reshape(9, BL)
            only_x[b, 1:5] = ns72[0:4, lb]
            only_y[b, 1:5] = ns72[4:8, lb]
            zerosum[b] = ns72[8, lb]

    cx = np.stack([(ip == p).sum(1) for p in range(5)], 1)  # [B,5]
    cy = np.stack([(op == p).sum(1) for p in range(5)], 1)

    loss_nonzero = np.float32(0.0)
    for p in range(1, 5):
        both = 0.5 * (sum_xy[:, p] / np.maximum(1, cy[:, p])
                      + sum_yx[:, p] / np.maximum(1, cx[:, p]))
        ox = only_x[:, p] / np.maximum(1, cx[:, p])
        oy = only_y[:, p] / np.maximum(1, cy[:, p])
        per_b = np.where(cy[:, p] == 0, ox, np.where(cx[:, p] == 0, oy, both))
        loss_nonzero = loss_nonzero + np.float32(per_b.mean())
    loss_zero = np.float32((zerosum / np.maximum(1, cy[:, 0])).mean())
    return np.float32(loss_nonzero), np.float32(loss_zero)


def kernel(target, reco, in_pid, out_pid):
    ip = np.asarray(in_pid)
    op = np.asarray(out_pid)
    # fixed group stride; bump (recompile) only if a pid group overflows it
    max_grp = 0
    for pid in (ip, op):
        for p in range(1, 5):
            max_grp = max(max_grp, int((pid == p).sum(1).max()))
    S = 130
    while S < max_grp:
        S += 8
    S = min(S, 512)  # a pid group can never exceed N=512

    nc = _get_program(S)
    in_maps = _prep_inputs(target, reco, ip, op, S)
    res = run_bass_kernel_spmd(nc, in_maps, list(range(NCORES)))
    sums_all = [res.results[ci]["sums"] for ci in range(NCORES)]
    ns_all = [res.results[ci]["ns"] for ci in range(NCORES)]
    return _epilogue(sums_all, ns_all, ip, op)



# revision 17
# speedup vs baseline: 1.7593x; 1.7593x over previous
"""Trainium2 Bass kernel for ChamferLossSplitPID.

Contract: kernel(**inputs) takes the FULL inputs (target/reco [64,512,4] f32,
in_pid/out_pid [64,512] i32) and returns the full output (loss_nonzero,
loss_zero) as float32 scalars, matching reference().

Strategy (8 NeuronCores, data-parallel over batch, 8 batches per core):
  The loss only needs distances between SAME-pid pairs, so instead of the
  full [N, N] distance matrix we compute only the 4 diagonal blocks of the
  pid-grouped matrix, in both directions: rows = points of pid p packed
  into a 128-partition chunk (zero-padded lhsT columns -> pad rows produce
  exactly 0, so no row masks are needed anywhere), cols = other-side points
  of pid p padded to a fixed S=130 stride (pad cols produce dist^2 = 2^27,
  never a min winner). dist^2 is a K=16 split-bf16 matmul (~1e-5 rel).
  Per (batch, dir) that is 4 matmuls of [16,128]x[16,S] -> one PSUM slot
  each; slots pack 3-per-bank so ONE 4D-AP DVE min-reduce drains a whole
  3-bank tile (9 slots). This cuts both PE and DVE work ~4x vs reducing
  the full matrix: the DVE (the bottleneck engine) reads 16*4*S instead of
  16*4*N+pads elements per partition. Rare pid groups with >128 members
  (P[size>128] ~ 0.2%) get their extra rows patched on the host in fp32.
  Tail per half: relu (DVE) -> sqrt (Act) -> ones-matmul partition sum
  (PE) -> Act copy -> DMA. Per-pid norm sums arrive host-premultiplied
  (norm*mask) and need only one DVE row-reduce. The tiny O(B*pid) epilogue
  (counts, divisions, empty-group branches, means) runs on the host, as
  does all layout prep (grouping, hi/lo splits, padding).

The emitted IR is input-value-independent (fixed group stride S, fixed
128-row chunks), so one SPMD program serves all 8 cores. S is bumped
automatically if some pid group exceeds it (recompile, still correct).
"""

import sys

sys.path.insert(0, "/opt/trn_rl_repo")

import numpy as np

from concourse import bacc, bass, mybir, tile
from concourse.bass_utils import run_bass_kernel_spmd

B, N, D = 64, 512, 4
NCORES = 8
BL = B // NCORES          # batches per core
P = 128                   # partitions
NPID = 4                  # nonzero pid classes
BIG = float(2 ** 27)      # pad-column dist^2 (exact in bf16)
KROWS = 16                # split-bf16 contraction rows
NBD = 2 * BL              # (batch, dir) pairs per core
NSLOT = NBD * NPID        # diagonal blocks per core (64)
F32 = mybir.dt.float32
BF16 = mybir.dt.bfloat16

_PROGRAM_CACHE = {}
_SKIP = set()   # debug: subset of {"tailb_mm", "tailb_copy", "taila", "norm"}


def _plan_tiles(S):
    """PSUM tile schedule: (start_slot, nslots) with a small ramp so the
    first DVE reduce starts after only a few matmuls. Slots are 256-element
    (1KB) regions, two per PSUM bank; a full 8-slot tile is 4 banks."""
    plan = []
    t0 = 0
    ramp = [4, 4]             # slots for the first tiles, then 8-slot tiles
    i = 0
    while t0 < NSLOT:
        ns = min(ramp[i] if i < len(ramp) else 8, NSLOT - t0)
        plan.append((t0, ns))
        t0 += ns
        i += 1
    return plan


def _build_program(S: int):
    """Emit the SPMD Bass program for group stride S. Value-independent."""
    COLS = NPID * S
    W = NPID * P + COLS       # lhsT block then rhs block, per direction
    plan = _plan_tiles(S)
    nc = bacc.Bacc(None)

    # lhsT and rhs for one (dir, batch) share one tensor/DMA so each
    # consuming Matmult carries a single sync wait (PE LW allows only one).
    d_ab = [[nc.dram_tensor(f"ab{d}_{b}", [KROWS, W], BF16, kind="ExternalInput")
             for b in range(BL)] for d in range(2)]
    # norm*mask premultiplied on host: row g*BL+b, g in (p1..p4 of in_pid,
    # p1..p4 of out_pid, p0 of out_pid)
    d_nrm = nc.dram_tensor("nrm", [9 * BL, N], F32, kind="ExternalInput")
    d_sums = nc.dram_tensor("sums", [1, NSLOT], F32, kind="ExternalOutput")
    d_ns = nc.dram_tensor("ns", [9 * BL, 1], F32, kind="ExternalOutput")

    with tile.TileContext(nc) as tc:
        with (
            tc.tile_pool(name="const", bufs=1) as const,
            tc.tile_pool(name="work", bufs=2) as work,
            tc.tile_pool(name="psum", bufs=2, space=bass.MemorySpace.PSUM) as psum,
        ):
            tAB = [[const.tile([KROWS, W], BF16, tag=f"ab{d}_{b}",
                               name=f"tAB{d}_{b}") for b in range(BL)]
                   for d in range(2)]
            # dir-0 batches split across the Sync and Act HWDGE queues; the
            # later-needed dir-1 batches (plus norms) go via the Pool HWDGE
            # (cheap 25ns queue config, GpSimd is otherwise idle).
            tNRM = const.tile([9 * BL, N], F32, tag="nrm")
            nc.gpsimd.dma_start(tNRM[:], d_nrm[:])
            for b in range(BL):
                eng = nc.sync if b % 2 == 0 else nc.scalar
                eng.dma_start(tAB[0][b][:], d_ab[0][b][:])
            for b in range(BL):
                nc.gpsimd.dma_start(tAB[1][b][:], d_ab[1][b][:])
            tONE = const.tile([P, 1], F32, tag="one")
            nc.vector.memset(tONE[:], 1.0)

            tMS = const.tile([P, NSLOT], F32, tag="ms")   # per-block minima
            tSQ = const.tile([P, NSLOT], F32, tag="sq")   # sqrt'd minima
            tSF = const.tile([1, NSLOT], F32, tag="sf")   # partition sums
            tNS = work.tile([9 * BL, 1], F32, tag="nsout")

            def emit_tile(t0, ns):
                # slot s = dir*32 + batch*4 + group (dir-major: the whole
                # first half only needs the dir-0 DMAs). Slots sit at a
                # uniform 256-element stride so ONE 3D-AP DVE reduce drains
                # the whole tile (the baseline-proven access shape).
                pt = psum.tile([P, ns, 256], F32, tag="dist")
                for i in range(ns):
                    s = t0 + i
                    dr, rem = divmod(s, BL * NPID)
                    b, g = divmod(rem, NPID)
                    nc.tensor.matmul(
                        pt[:, i, 0:S],
                        tAB[dr][b][:, g * P : (g + 1) * P],
                        tAB[dr][b][:, NPID * P + g * S : NPID * P + (g + 1) * S],
                        start=True,
                        stop=True,
                    )
                nc.vector.tensor_reduce(
                    tMS[:, t0 : t0 + ns],
                    pt[:, :, 0:S],
                    axis=mybir.AxisListType.X,
                    op=mybir.AluOpType.min,
                )

            def tail_a(lo, hi):
                # pad rows are exactly 0 (zero lhsT cols) and real minima are
                # far from 0, so relu only guards fp rounding; sqrt(0)=0 means
                # pad rows drop out of the partition sums without any mask.
                if "taila" in _SKIP:
                    return
                nc.vector.tensor_scalar_max(tMS[:, lo:hi], tMS[:, lo:hi], 0.0)
                nc.scalar.activation(
                    tSQ[:, lo:hi], tMS[:, lo:hi], mybir.ActivationFunctionType.Sqrt)

            def tail_b(lo, hi):
                if "tailb_mm" in _SKIP:
                    nc.vector.tensor_scalar_max(tSF[:, lo:hi], tSQ[0:1, lo:hi], 0.0)
                    nc.sync.dma_start(d_sums[:, lo:hi], tSF[:, lo:hi])
                    return
                po = psum.tile([1, hi - lo], F32, tag="dist", name=f"po{lo}")
                nc.tensor.matmul(po[:], tONE[:], tSQ[:, lo:hi], start=True, stop=True)
                if "tailb_copy" in _SKIP:
                    nc.vector.tensor_scalar_max(tSF[:, lo:hi], po[:], -1e30)
                else:
                    nc.scalar.copy(tSF[:, lo:hi], po[:])
                nc.sync.dma_start(d_sums[:, lo:hi], tSF[:, lo:hi])

            # piece boundary at the first tile edge covering half the slots
            acc, half = 0, NSLOT
            for t0, ns in plan:
                acc += ns
                if acc >= NSLOT // 2:
                    half = acc
                    break

            emitted = 0
            pending = []  # deferred tail_b stages: (emit_after_slots, lo, hi)
            for idx, (t0, ns) in enumerate(plan):
                emit_tile(t0, ns)
                emitted += ns
                if emitted == half and emitted < NSLOT:
                    tail_a(0, half)
                    # defer the PE/Act stage ~2 tiles so the PE never stalls
                    pending.append((min(NSLOT, emitted + 18), 0, half))
                if idx == 1 and "norm" not in _SKIP:
                    # norm-sum reduce early, during the DVE pipeline ramp
                    nc.vector.tensor_reduce(
                        tNS[:], tNRM[:], axis=mybir.AxisListType.X,
                        op=mybir.AluOpType.add)
                    nc.sync.dma_start(d_ns[:], tNS[:])
                while pending and emitted >= pending[0][0]:
                    _, lo, hi = pending.pop(0)
                    tail_b(lo, hi)
            tail_a(half, NSLOT)
            for _, lo, hi in pending:
                tail_b(lo, hi)
            tail_b(half, NSLOT)

    nc.compile()
    return nc


def _get_program(S: int):
    if S not in _PROGRAM_CACHE:
        _PROGRAM_CACHE[S] = _build_program(S)
    return _PROGRAM_CACHE[S]


def _prep_inputs(target, reco, in_pid, out_pid, S):
    """Build per-core input maps. All heavy compute stays on device; this is
    O(B*N) metadata/layout prep (grouping, norms, hi/lo splits, padding)."""
    COLS = NPID * S
    W = NPID * P + COLS
    t = np.ascontiguousarray(np.asarray(target, dtype=np.float32))
    r = np.ascontiguousarray(np.asarray(reco, dtype=np.float32))
    ip = np.asarray(in_pid)
    op = np.asarray(out_pid)

    import ml_dtypes

    def split16(x):
        hi = x.astype(ml_dtypes.bfloat16).astype(np.float32)
        lo = (x - hi).astype(ml_dtypes.bfloat16).astype(np.float32)
        return hi, lo

    nt2 = (t * t).sum(-1)                      # [B,N]
    nr2 = (r * r).sum(-1)
    ones = np.ones((B, 1, N), np.float32)
    # split-bf16: a.b ~= ahi.bhi + ahi.blo + alo.bhi (lo.lo dropped, ~2^-16 rel)
    # lhsT rows: [(-2x)hi x4, (-2x)hi x4, (-2x)lo x4, |x|2hi, |x|2lo, 1, 1]
    # rhs rows:  [ yhi x4,     ylo x4,     yhi x4,    1,      1, |y|2hi, |y|2lo]
    def build_lhs(x, x2):
        m2hi, m2lo = split16(-2.0 * x.transpose(0, 2, 1))   # [B,4,N]
        x2hi, x2lo = split16(x2[:, None, :])                # [B,1,N]
        return np.concatenate(
            [m2hi, m2hi, m2lo, x2hi, x2lo, ones, ones], axis=1)  # [B,16,N]

    Lt = build_lhs(t, nt2)
    Lr = build_lhs(r, nr2)
    thi, tlo = split16(t)
    rhi, rlo = split16(r)
    t2hi, t2lo = split16(nt2)
    r2hi, r2lo = split16(nr2)

    AB = np.zeros((2, B, KROWS, W), np.float32)
    sides = [(Lt, ip, rhi, rlo, r2hi, r2lo, op),   # dir0: rows targets, cols recos
             (Lr, op, thi, tlo, t2hi, t2lo, ip)]   # dir1: rows recos, cols targets
    for dirn, (xL, xpid, yhi, ylo, y2hi, y2lo, ypid) in enumerate(sides):
        for b in range(B):
            for g in range(NPID):
                p = g + 1
                ridx = np.nonzero(xpid[b] == p)[0][:P]
                AB[dirn, b, :, g * P : g * P + len(ridx)] = xL[b][:, ridx]
                cidx = np.nonzero(ypid[b] == p)[0]
                c0 = NPID * P + g * S
                k = len(cidx)
                AB[dirn, b, 0:4, c0 : c0 + k] = yhi[b, cidx].T
                AB[dirn, b, 4:8, c0 : c0 + k] = ylo[b, cidx].T
                AB[dirn, b, 8:12, c0 : c0 + k] = yhi[b, cidx].T
                AB[dirn, b, 12:14, c0 : c0 + k] = 1.0
                AB[dirn, b, 14, c0 : c0 + k] = y2hi[b, cidx]
                AB[dirn, b, 15, c0 : c0 + k] = y2lo[b, cidx]
                AB[dirn, b, 14, c0 + k : c0 + S] = BIG

    normt = np.sqrt(nt2).astype(np.float32)
    normr = np.sqrt(nr2).astype(np.float32)
    # 72-row layout, premultiplied: row g*BL+b; g 0-3: normt*(in_pid==g+1);
    # g 4-7: normr*(out_pid==g-3); g 8: normr*(out_pid==0)
    grp = [normt * (ip == p) for p in (1, 2, 3, 4)]
    grp += [normr * (op == p) for p in (1, 2, 3, 4, 0)]

    in_maps = []
    for ci in range(NCORES):
        s = slice(ci * BL, (ci + 1) * BL)
        m = {"nrm": np.ascontiguousarray(
            np.concatenate([g[s] for g in grp], axis=0))}
        for dirn in range(2):
            for b in range(BL):
                m[f"ab{dirn}_{b}"] = np.ascontiguousarray(
                    AB[dirn, ci * BL + b].astype(ml_dtypes.bfloat16))
        in_maps.append(m)
    return in_maps


def _overflow_corrections(t, r, ip, op):
    """fp32 host patch for pid groups with >128 members: device rows are
    capped at 128 partitions, the few extra rows' min-distances are added
    here. O(overflow_rows * S) — expected ~zero rows per input."""
    corr_xy = np.zeros((B, 5))
    corr_yx = np.zeros((B, 5))
    for b in range(B):
        for p in range(1, 5):
            ridx = np.nonzero(ip[b] == p)[0]
            cidx = np.nonzero(op[b] == p)[0]
            if len(ridx) > P and len(cidx) > 0:
                for i in ridx[P:]:
                    d2 = ((t[b, i][None, :] - r[b, cidx]) ** 2).sum(-1)
                    corr_xy[b, p] += np.sqrt(d2.min())
            if len(cidx) > P and len(ridx) > 0:
                for j in cidx[P:]:
                    d2 = ((r[b, j][None, :] - t[b, ridx]) ** 2).sum(-1)
                    corr_yx[b, p] += np.sqrt(d2.min())
    return corr_xy, corr_yx


def _epilogue(sums_all, ns_all, t, r, ip, op):
    """Tiny O(B*pid) final combination, mirrors reference()'s branch logic."""
    sum_xy = np.zeros((B, 5))
    sum_yx = np.zeros((B, 5))
    only_x = np.zeros((B, 5))
    only_y = np.zeros((B, 5))
    zerosum = np.zeros(B)
    for ci in range(NCORES):
        srow = sums_all[ci].reshape(2, BL, NPID)   # slot = dir*32 + b*4 + g
        ns72 = ns_all[ci].reshape(9, BL)
        for lb in range(BL):
            b = ci * BL + lb
            sum_xy[b, 1:5] = srow[0, lb]
            sum_yx[b, 1:5] = srow[1, lb]
            only_x[b, 1:5] = ns72[0:4, lb]
            only_y[b, 1:5] = ns72[4:8, lb]
            zerosum[b] = ns72[8, lb]

    cxy, cyx = _overflow_corrections(t, r, ip, op)
    sum_xy += cxy
    sum_yx += cyx

    cx = np.stack([(ip == p).sum(1) for p in range(5)], 1)  # [B,5]
    cy = np.stack([(op == p).sum(1) for p in range(5)], 1)

    loss_nonzero = np.float32(0.0)
    for p in range(1, 5):
        both = 0.5 * (sum_xy[:, p] / np.maximum(1, cy[:, p])
                      + sum_yx[:, p] / np.maximum(1, cx[:, p]))
        ox = only_x[:, p] / np.maximum(1, cx[:, p])
        oy = only_y[:, p] / np.maximum(1, cy[:, p])
        per_b = np.where(cy[:, p] == 0, ox, np.where(cx[:, p] == 0, oy, both))
        loss_nonzero = loss_nonzero + np.float32(per_b.mean())
    loss_zero = np.float32((zerosum / np.maximum(1, cy[:, 0])).mean())
    return np.float32(loss_nonzero), np.float32(loss_zero)


def kernel(target, reco, in_pid, out_pid):
    t = np.ascontiguousarray(np.asarray(target, dtype=np.float32))
    r = np.ascontiguousarray(np.asarray(reco, dtype=np.float32))
    ip = np.asarray(in_pid)
    op = np.asarray(out_pid)
    # fixed group stride; bump (recompile) only if a pid group overflows it
    max_grp = 0
    for pid in (ip, op):
        for p in range(1, 5):
            max_grp = max(max_grp, int((pid == p).sum(1).max()))
    S = 130
    while S < max_grp:
        S += 8
    S = min(S, 512)  # a pid group can never exceed N=512

    nc = _get_program(S)
    in_maps = _prep_inputs(t, r, ip, op, S)
    res = run_bass_kernel_spmd(nc, in_maps, list(range(NCORES)))
    sums_all = [res.results[ci]["sums"] for ci in range(NCORES)]
    ns_all = [res.results[ci]["ns"] for ci in range(NCORES)]
    return _epilogue(sums_all, ns_all, t, r, ip, op)


# revision 19
# speedup vs baseline: 1.8904x; 1.0745x over previous
"""Trainium2 Bass kernel for ChamferLossSplitPID.

Contract: kernel(**inputs) takes the FULL inputs (target/reco [64,512,4] f32,
in_pid/out_pid [64,512] i32) and returns the full output (loss_nonzero,
loss_zero) as float32 scalars, matching reference().

Strategy (8 NeuronCores, data-parallel over batch, 8 batches per core):
  The loss only needs distances between SAME-pid pairs, so instead of the
  full [N, N] distance matrix we compute only the 4 diagonal blocks of the
  pid-grouped matrix, in both directions: rows = points of pid p packed
  into a 128-partition chunk (zero-padded lhsT columns -> pad rows produce
  exactly 0, so no row masks are needed anywhere), cols = other-side points
  of pid p padded to a fixed S=130 stride (pad cols produce dist^2 = 2^27,
  never a min winner). dist^2 is a K=16 split-bf16 matmul (~1e-5 rel).
  Per (batch, dir) that is 4 matmuls of [16,128]x[16,S] -> one PSUM slot
  each; slots pack 3-per-bank so ONE 4D-AP DVE min-reduce drains a whole
  3-bank tile (9 slots). This cuts both PE and DVE work ~4x vs reducing
  the full matrix: the DVE (the bottleneck engine) reads 16*4*S instead of
  16*4*N+pads elements per partition. Rare pid groups with >128 members
  (P[size>128] ~ 0.2%) get their extra rows patched on the host in fp32.
  Tail per half: relu (DVE) -> sqrt (Act) -> ones-matmul partition sum
  (PE) -> Act copy -> DMA. Per-pid norm sums arrive host-premultiplied
  (norm*mask) and need only one DVE row-reduce. The tiny O(B*pid) epilogue
  (counts, divisions, empty-group branches, means) runs on the host, as
  does all layout prep (grouping, hi/lo splits, padding).

The emitted IR is input-value-independent (fixed group stride S, fixed
128-row chunks), so one SPMD program serves all 8 cores. S is bumped
automatically if some pid group exceeds it (recompile, still correct).
"""

import sys

sys.path.insert(0, "/opt/trn_rl_repo")

import numpy as np

from concourse import bacc, bass, mybir, tile
from concourse.bass_utils import run_bass_kernel_spmd

B, N, D = 64, 512, 4
NCORES = 8
BL = B // NCORES          # batches per core
P = 128                   # partitions
NPID = 4                  # nonzero pid classes
BIG = float(2 ** 27)      # pad-column dist^2 (exact in bf16)
KROWS = 16                # split-bf16 contraction rows
NBD = 2 * BL              # (batch, dir) pairs per core
NSLOT = NBD * NPID        # diagonal blocks per core (64)
F32 = mybir.dt.float32
BF16 = mybir.dt.bfloat16

_PROGRAM_CACHE = {}
_SKIP = set()   # debug: subset of {"tailb_mm", "tailb_copy", "taila", "norm"}


def _plan_tiles(S):
    """PSUM tile schedule: (start_slot, nslots) with a small ramp so the
    first DVE reduce starts after only a few matmuls. Slots are 256-element
    (1KB) regions, two per PSUM bank; a full 8-slot tile is 4 banks."""
    plan = []
    t0 = 0
    ramp = [4, 4]             # slots for the first tiles, then 8-slot tiles
    i = 0
    while t0 < NSLOT:
        ns = min(ramp[i] if i < len(ramp) else 8, NSLOT - t0)
        plan.append((t0, ns))
        t0 += ns
        i += 1
    return plan


def _build_program(S: int):
    """Emit the SPMD Bass program for group stride S. Value-independent."""
    COLS = NPID * S
    W = NPID * P + COLS       # lhsT block then rhs block, per direction
    plan = _plan_tiles(S)
    nc = bacc.Bacc(None)

    # lhsT and rhs for one (dir, batch) share one tensor/DMA so each
    # consuming Matmult carries a single sync wait (PE LW allows only one).
    d_ab = [[nc.dram_tensor(f"ab{d}_{b}", [KROWS, W], BF16, kind="ExternalInput")
             for b in range(BL)] for d in range(2)]
    # norm*mask premultiplied on host: row g*BL+b, g in (p1..p4 of in_pid,
    # p1..p4 of out_pid, p0 of out_pid)
    d_nrm = nc.dram_tensor("nrm", [9 * BL, N], F32, kind="ExternalInput")
    d_sums = nc.dram_tensor("sums", [1, NSLOT], F32, kind="ExternalOutput")
    d_ns = nc.dram_tensor("ns", [9 * BL, 1], F32, kind="ExternalOutput")

    with tile.TileContext(nc) as tc:
        with (
            tc.tile_pool(name="const", bufs=1) as const,
            tc.tile_pool(name="work", bufs=2) as work,
            tc.tile_pool(name="psum", bufs=2, space=bass.MemorySpace.PSUM) as psum,
        ):
            tAB = [[const.tile([KROWS, W], BF16, tag=f"ab{d}_{b}",
                               name=f"tAB{d}_{b}") for b in range(BL)]
                   for d in range(2)]
            # interleave batches across the Sync and Act HWDGE queues in
            # consumption order: dir-0 batches first, then dir-1, then norms
            # (only needed once the DVE pipeline is already running).
            tNRM = const.tile([9 * BL, N], F32, tag="nrm")
            for dr in range(2):
                for b in range(BL):
                    eng = nc.sync if b % 2 == 0 else nc.scalar
                    eng.dma_start(tAB[dr][b][:], d_ab[dr][b][:])
            nc.scalar.dma_start(tNRM[:], d_nrm[:])
            tONE = const.tile([P, 1], F32, tag="one")
            nc.vector.memset(tONE[:], 1.0)

            tMS = const.tile([P, NSLOT], F32, tag="ms")   # per-block minima
            tSQ = const.tile([P, NSLOT], F32, tag="sq")   # sqrt'd minima
            tSF = const.tile([1, NSLOT], F32, tag="sf")   # partition sums
            tNS = work.tile([9 * BL, 1], F32, tag="nsout")

            def emit_tile(t0, ns):
                # slot s = dir*32 + batch*4 + group (dir-major: the whole
                # first half only needs the dir-0 DMAs). Slots sit at a
                # uniform 256-element stride so ONE 3D-AP DVE reduce drains
                # the whole tile (the baseline-proven access shape).
                pt = psum.tile([P, ns, 256], F32, tag="dist")
                for i in range(ns):
                    s = t0 + i
                    dr, rem = divmod(s, BL * NPID)
                    b, g = divmod(rem, NPID)
                    nc.tensor.matmul(
                        pt[:, i, 0:S],
                        tAB[dr][b][:, g * P : (g + 1) * P],
                        tAB[dr][b][:, NPID * P + g * S : NPID * P + (g + 1) * S],
                        start=True,
                        stop=True,
                    )
                nc.vector.tensor_reduce(
                    tMS[:, t0 : t0 + ns],
                    pt[:, :, 0:S],
                    axis=mybir.AxisListType.X,
                    op=mybir.AluOpType.min,
                )

            def tail_a(lo, hi):
                # pad rows are exactly 0 (zero lhsT cols) and real minima are
                # far from 0, so relu only guards fp rounding; sqrt(0)=0 means
                # pad rows drop out of the partition sums without any mask.
                if "taila" in _SKIP:
                    return
                nc.vector.tensor_scalar_max(tMS[:, lo:hi], tMS[:, lo:hi], 0.0)
                nc.scalar.activation(
                    tSQ[:, lo:hi], tMS[:, lo:hi], mybir.ActivationFunctionType.Sqrt)

            def tail_b(lo, hi):
                if "tailb_mm" in _SKIP:
                    nc.vector.tensor_scalar_max(tSF[:, lo:hi], tSQ[0:1, lo:hi], 0.0)
                    nc.sync.dma_start(d_sums[:, lo:hi], tSF[:, lo:hi])
                    return
                po = psum.tile([1, hi - lo], F32, tag="dist", name=f"po{lo}")
                nc.tensor.matmul(po[:], tONE[:], tSQ[:, lo:hi], start=True, stop=True)
                if "tailb_copy" in _SKIP:
                    nc.vector.tensor_scalar_max(tSF[:, lo:hi], po[:], -1e30)
                else:
                    nc.scalar.copy(tSF[:, lo:hi], po[:])
                nc.sync.dma_start(d_sums[:, lo:hi], tSF[:, lo:hi])

            # piece boundary at the first tile edge covering half the slots
            acc, half = 0, NSLOT
            for t0, ns in plan:
                acc += ns
                if acc >= NSLOT // 2:
                    half = acc
                    break

            emitted = 0
            pending = []  # deferred tail_b stages: (emit_after_slots, lo, hi)
            for idx, (t0, ns) in enumerate(plan):
                emit_tile(t0, ns)
                emitted += ns
                if emitted == half and emitted < NSLOT:
                    tail_a(0, half)
                    # defer the PE/Act stage ~2 tiles so the PE never stalls
                    pending.append((min(NSLOT, emitted + 18), 0, half))
                if idx == 5 and "norm" not in _SKIP:
                    # norm-sum reduce early, during the DVE pipeline ramp
                    nc.vector.tensor_reduce(
                        tNS[:], tNRM[:], axis=mybir.AxisListType.X,
                        op=mybir.AluOpType.add)
                    nc.sync.dma_start(d_ns[:], tNS[:])
                while pending and emitted >= pending[0][0]:
                    _, lo, hi = pending.pop(0)
                    tail_b(lo, hi)
            tail_a(half, NSLOT)
            for _, lo, hi in pending:
                tail_b(lo, hi)
            tail_b(half, NSLOT)

    nc.compile()
    return nc


def _get_program(S: int):
    if S not in _PROGRAM_CACHE:
        _PROGRAM_CACHE[S] = _build_program(S)
    return _PROGRAM_CACHE[S]


def _prep_inputs(target, reco, in_pid, out_pid, S):
    """Build per-core input maps. All heavy compute stays on device; this is
    O(B*N) metadata/layout prep (grouping, norms, hi/lo splits, padding)."""
    COLS = NPID * S
    W = NPID * P + COLS
    t = np.ascontiguousarray(np.asarray(target, dtype=np.float32))
    r = np.ascontiguousarray(np.asarray(reco, dtype=np.float32))
    ip = np.asarray(in_pid)
    op = np.asarray(out_pid)

    import ml_dtypes

    def split16(x):
        hi = x.astype(ml_dtypes.bfloat16).astype(np.float32)
        lo = (x - hi).astype(ml_dtypes.bfloat16).astype(np.float32)
        return hi, lo

    nt2 = (t * t).sum(-1)                      # [B,N]
    nr2 = (r * r).sum(-1)
    ones = np.ones((B, 1, N), np.float32)
    # split-bf16: a.b ~= ahi.bhi + ahi.blo + alo.bhi (lo.lo dropped, ~2^-16 rel)
    # lhsT rows: [(-2x)hi x4, (-2x)hi x4, (-2x)lo x4, |x|2hi, |x|2lo, 1, 1]
    # rhs rows:  [ yhi x4,     ylo x4,     yhi x4,    1,      1, |y|2hi, |y|2lo]
    def build_lhs(x, x2):
        m2hi, m2lo = split16(-2.0 * x.transpose(0, 2, 1))   # [B,4,N]
        x2hi, x2lo = split16(x2[:, None, :])                # [B,1,N]
        return np.concatenate(
            [m2hi, m2hi, m2lo, x2hi, x2lo, ones, ones], axis=1)  # [B,16,N]

    Lt = build_lhs(t, nt2)
    Lr = build_lhs(r, nr2)
    thi, tlo = split16(t)
    rhi, rlo = split16(r)
    t2hi, t2lo = split16(nt2)
    r2hi, r2lo = split16(nr2)

    AB = np.zeros((2, B, KROWS, W), np.float32)
    sides = [(Lt, ip, rhi, rlo, r2hi, r2lo, op),   # dir0: rows targets, cols recos
             (Lr, op, thi, tlo, t2hi, t2lo, ip)]   # dir1: rows recos, cols targets
    for dirn, (xL, xpid, yhi, ylo, y2hi, y2lo, ypid) in enumerate(sides):
        for b in range(B):
            for g in range(NPID):
                p = g + 1
                ridx = np.nonzero(xpid[b] == p)[0][:P]
                AB[dirn, b, :, g * P : g * P + len(ridx)] = xL[b][:, ridx]
                cidx = np.nonzero(ypid[b] == p)[0]
                c0 = NPID * P + g * S
                k = len(cidx)
                AB[dirn, b, 0:4, c0 : c0 + k] = yhi[b, cidx].T
                AB[dirn, b, 4:8, c0 : c0 + k] = ylo[b, cidx].T
                AB[dirn, b, 8:12, c0 : c0 + k] = yhi[b, cidx].T
                AB[dirn, b, 12:14, c0 : c0 + k] = 1.0
                AB[dirn, b, 14, c0 : c0 + k] = y2hi[b, cidx]
                AB[dirn, b, 15, c0 : c0 + k] = y2lo[b, cidx]
                AB[dirn, b, 14, c0 + k : c0 + S] = BIG

    normt = np.sqrt(nt2).astype(np.float32)
    normr = np.sqrt(nr2).astype(np.float32)
    # 72-row layout, premultiplied: row g*BL+b; g 0-3: normt*(in_pid==g+1);
    # g 4-7: normr*(out_pid==g-3); g 8: normr*(out_pid==0)
    grp = [normt * (ip == p) for p in (1, 2, 3, 4)]
    grp += [normr * (op == p) for p in (1, 2, 3, 4, 0)]

    in_maps = []
    for ci in range(NCORES):
        s = slice(ci * BL, (ci + 1) * BL)
        m = {"nrm": np.ascontiguousarray(
            np.concatenate([g[s] for g in grp], axis=0))}
        for dirn in range(2):
            for b in range(BL):
                m[f"ab{dirn}_{b}"] = np.ascontiguousarray(
                    AB[dirn, ci * BL + b].astype(ml_dtypes.bfloat16))
        in_maps.append(m)
    return in_maps


def _overflow_corrections(t, r, ip, op):
    """fp32 host patch for pid groups with >128 members: device rows are
    capped at 128 partitions, the few extra rows' min-distances are added
    here. O(overflow_rows * S) — expected ~zero rows per input."""
    corr_xy = np.zeros((B, 5))
    corr_yx = np.zeros((B, 5))
    for b in range(B):
        for p in range(1, 5):
            ridx = np.nonzero(ip[b] == p)[0]
            cidx = np.nonzero(op[b] == p)[0]
            if len(ridx) > P and len(cidx) > 0:
                for i in ridx[P:]:
                    d2 = ((t[b, i][None, :] - r[b, cidx]) ** 2).sum(-1)
                    corr_xy[b, p] += np.sqrt(d2.min())
            if len(cidx) > P and len(ridx) > 0:
                for j in cidx[P:]:
                    d2 = ((r[b, j][None, :] - t[b, ridx]) ** 2).sum(-1)
                    corr_yx[b, p] += np.sqrt(d2.min())
    return corr_xy, corr_yx


def _epilogue(sums_all, ns_all, t, r, ip, op):
    """Tiny O(B*pid) final combination, mirrors reference()'s branch logic."""
    sum_xy = np.zeros((B, 5))
    sum_yx = np.zeros((B, 5))
    only_x = np.zeros((B, 5))
    only_y = np.zeros((B, 5))
    zerosum = np.zeros(B)
    for ci in range(NCORES):
        srow = sums_all[ci].reshape(2, BL, NPID)   # slot = dir*32 + b*4 + g
        ns72 = ns_all[ci].reshape(9, BL)
        for lb in range(BL):
            b = ci * BL + lb
            sum_xy[b, 1:5] = srow[0, lb]
            sum_yx[b, 1:5] = srow[1, lb]
            only_x[b, 1:5] = ns72[0:4, lb]
            only_y[b, 1:5] = ns72[4:8, lb]
            zerosum[b] = ns72[8, lb]

    cxy, cyx = _overflow_corrections(t, r, ip, op)
    sum_xy += cxy
    sum_yx += cyx

    cx = np.stack([(ip == p).sum(1) for p in range(5)], 1)  # [B,5]
    cy = np.stack([(op == p).sum(1) for p in range(5)], 1)

    loss_nonzero = np.float32(0.0)
    for p in range(1, 5):
        both = 0.5 * (sum_xy[:, p] / np.maximum(1, cy[:, p])
                      + sum_yx[:, p] / np.maximum(1, cx[:, p]))
        ox = only_x[:, p] / np.maximum(1, cx[:, p])
        oy = only_y[:, p] / np.maximum(1, cy[:, p])
        per_b = np.where(cy[:, p] == 0, ox, np.where(cx[:, p] == 0, oy, both))
        loss_nonzero = loss_nonzero + np.float32(per_b.mean())
    loss_zero = np.float32((zerosum / np.maximum(1, cy[:, 0])).mean())
    return np.float32(loss_nonzero), np.float32(loss_zero)


def kernel(target, reco, in_pid, out_pid):
    t = np.ascontiguousarray(np.asarray(target, dtype=np.float32))
    r = np.ascontiguousarray(np.asarray(reco, dtype=np.float32))
    ip = np.asarray(in_pid)
    op = np.asarray(out_pid)
    # fixed group stride; bump (recompile) only if a pid group overflows it
    max_grp = 0
    for pid in (ip, op):
        for p in range(1, 5):
            max_grp = max(max_grp, int((pid == p).sum(1).max()))
    S = 130
    while S < max_grp:
        S += 8
    S = min(S, 512)  # a pid group can never exceed N=512

    nc = _get_program(S)
    in_maps = _prep_inputs(t, r, ip, op, S)
    res = run_bass_kernel_spmd(nc, in_maps, list(range(NCORES)))
    sums_all = [res.results[ci]["sums"] for ci in range(NCORES)]
    ns_all = [res.results[ci]["ns"] for ci in range(NCORES)]
    return _epilogue(sums_all, ns_all, t, r, ip, op)
